# revision 1
# baseline (speedup 1.0000x reference)
"""Trainium2 Bass kernel for the SE-sweep DAG-RNN (nn_DAG_RNN_se).

Reference semantics (B=32, C=512, H=W=32):
    h[i,j] = relu(x[:,:,i,j] + (h[i-1,j] + h[i,j-1]) @ W_hh)     # [B, C]
    y[i,j] = h[i,j] @ W_yh + bias

Strategy:
  * Data-parallel over batch: 8 cores x 4 batch elements, zero communication.
  * Anti-diagonal wavefront inside a core: diagonal d holds n_d cells
    (n_d = min(d,31)-max(0,d-31)+1); all cells of a diagonal are batched
    into one set of matmuls.
  * State layout is transposed: h^T tiles [C(4x128 partitions), n_d*B_local]
    so the recurrent matmuls keep W_hh chunks as the stationary operand:
       psum[c_out_chunk] += W_hh[k,cout]^chunk.T-free @ hs[k]
    16 matmuls (4 k-chunks x 4 cout-chunks) per diagonal, N = 4*n_d <= 128.
  * h_sum (left+up neighbour sum) is a free-dim shifted add (cells on a
    diagonal ordered by row index: neighbours of slot s on diag d+1 are
    slots s/s-1 (expanding) or s/s+1 (contracting) of diag d).
  * fp16 state + weights for the matmuls (1 cyc/row on PE like bf16, but
    ~8x finer mantissa); x-add and PSUM accumulate in fp32.
  * Output transform y = h @ W_yh done in [128,512]-wide matmuls per
    512-column chunk of the hidden buffer, interleaved into PE bubbles of
    the wavefront recurrence as chunks complete.

The full (unsharded) numpy contract lives in `kernel(**inputs)` below; the
Bass program is built and compiled once and cached at module level.
"""

import sys

if "/opt/trn_rl_repo" not in sys.path:
    sys.path.insert(0, "/opt/trn_rl_repo")

import numpy as np

import concourse.bass as bass
import concourse.mybir as mybir
import concourse.tile as tile
from concourse import bacc
from concourse import bass_utils

# ---------------------------------------------------------------- constants
B, C, H, W = 32, 512, 32, 32
NCORES = 8
BL = B // NCORES            # local batch per core = 4
ND = H + W - 1              # 63 diagonals
CT = 4                      # channel chunks of 128
P = 128
YCH = 8                     # output column chunks of 512

F32 = mybir.dt.float32
F16 = mybir.dt.float16
ALU = mybir.AluOpType
ACTF = mybir.ActivationFunctionType

N_D = [min(d, H - 1) - max(0, d - (W - 1)) + 1 for d in range(ND)]
IMIN = [max(0, d - (W - 1)) for d in range(ND)]
OFFB = [0] * (ND + 1)
for _d in range(ND):
    OFFB[_d + 1] = OFFB[_d] + N_D[_d] * BL
TOT = OFFB[ND]              # 4096 columns per chunk row

# diag after which output chunk ch (cols [512ch, 512ch+512)) is complete
YREADY = [min(d for d in range(ND) if OFFB[d + 1] >= 512 * (ch + 1))
          for ch in range(YCH)]
YLAG = 2


def _build_program():
    nc = bacc.Bacc("TRN2", target_bir_lowering=False, debug=False,
                   num_devices=NCORES)

    xs = nc.dram_tensor("xs", [P, CT * TOT], F32, kind="ExternalInput").ap()
    whh = nc.dram_tensor("whh", [C, C], F16, kind="ExternalInput").ap()
    wyh = nc.dram_tensor("wyh", [C, C], F16, kind="ExternalInput").ap()
    biasp = nc.dram_tensor("biasp", [P, CT], F32, kind="ExternalInput").ap()
    y = nc.dram_tensor("y", [C, TOT], F32, kind="ExternalOutput").ap()

    with tile.TileContext(nc) as tc:
        with (
            tc.tile_pool(name="persist", bufs=1) as persist,
            tc.tile_pool(name="xpool", bufs=8) as xpool,
            tc.tile_pool(name="tmppool", bufs=6) as tmppool,
            tc.tile_pool(name="hspool", bufs=3) as hspool,
            tc.tile_pool(name="ypool", bufs=4) as ypool,
            tc.tile_pool(name="recps", bufs=6, space="PSUM") as recps,
            tc.tile_pool(name="yps", bufs=2, space="PSUM") as yps,
        ):
            # ---- resident weights / bias / hidden ----
            whh_sb = persist.tile([P, CT * C], F16, name="whh_sb")
            wyh_sb = persist.tile([P, CT * C], F16, name="wyh_sb")
            bias_sb = persist.tile([P, CT], F32, name="bias_sb")
            h_all = [persist.tile([P, TOT], F16, name=f"hall{k}")
                     for k in range(CT)]
            for k in range(CT):
                nc.sync.dma_start(whh_sb[:, k * C:(k + 1) * C],
                                  whh[k * P:(k + 1) * P, :])
                nc.sync.dma_start(wyh_sb[:, k * C:(k + 1) * C],
                                  wyh[k * P:(k + 1) * P, :])
            nc.sync.dma_start(bias_sb[:], biasp[:])

            def w_slice(wsb, k, ct):
                return wsb[:, k * C + ct * P: k * C + ct * P + P]

            y_emitted = [False] * YCH

            def emit_y_chunk(ch):
                for ct in range(CT):
                    psy = yps.tile([P, 512], F32, tag="psy", name=f"psy{ch}_{ct}")
                    for k in range(CT):
                        nc.tensor.matmul(
                            psy[:],
                            lhsT=w_slice(wyh_sb, k, ct),
                            rhs=h_all[k][:, ch * 512:(ch + 1) * 512],
                            start=(k == 0), stop=(k == CT - 1))
                    ysb = ypool.tile([P, 512], F32, tag="ysb", name=f"ysb{ch}_{ct}")
                    if (ch * CT + ct) % 2 == 0:
                        nc.vector.tensor_scalar_add(ysb[:], psy[:],
                                                    bias_sb[:, ct:ct + 1])
                    else:
                        nc.scalar.activation(ysb[:], psy[:], ACTF.Identity,
                                             bias=bias_sb[:, ct:ct + 1],
                                             scale=1.0)
                    nc.sync.dma_start(
                        y[ct * P:(ct + 1) * P, ch * 512:(ch + 1) * 512],
                        ysb[:])

            hs_prev = None
            for d in range(ND):
                n = N_D[d]
                N = n * BL
                xt = xpool.tile([P, CT * N], F32, tag="xt", name=f"xt{d}")
                nc.sync.dma_start(
                    xt[:], xs[:, CT * OFFB[d]: CT * OFFB[d] + CT * N])

                tmps = [None] * CT
                if d + 1 < ND:
                    n2 = N_D[d + 1]
                    hs_next = [hspool.tile([P, n2 * BL], F16, tag=f"hs{k}",
                                           name=f"hs{k}_{d + 1}")
                               for k in range(CT)]
                else:
                    hs_next = None

                for ct in range(CT):
                    hsl = h_all[ct][:, OFFB[d]: OFFB[d] + N]
                    if d == 0:
                        # h = relu(x) straight from the input tile
                        nc.scalar.activation(hsl, xt[:, ct * N:(ct + 1) * N],
                                             ACTF.Relu)
                    else:
                        ps = recps.tile([P, N], F32, tag="ps", name=f"ps{d}_{ct}")
                        korder = [(ct + 1) % CT, (ct + 2) % CT,
                                  (ct + 3) % CT, ct]
                        for idx, k in enumerate(korder):
                            nc.tensor.matmul(
                                ps[:],
                                lhsT=w_slice(whh_sb, k, ct),
                                rhs=hs_prev[k][:],
                                start=(idx == 0), stop=(idx == CT - 1))
                        tmp = tmppool.tile([P, N], F32, tag="tmp",
                                           name=f"tmp{d}_{ct}")
                        nc.vector.scalar_tensor_tensor(
                            out=tmp[:], in0=ps[:], scalar=0.0,
                            op0=ALU.add, op1=ALU.add,
                            in1=xt[:, ct * N:(ct + 1) * N])
                        nc.scalar.activation(hsl, tmp[:], ACTF.Relu)
                        tmps[ct] = tmp

                    # h_sum for diag d+1, chunk ct
                    if hs_next is not None:
                        hs = hs_next[ct]
                        tmp = tmps[ct]
                        if d + 1 <= W - 1:
                            # expanding: n2 = n+1
                            if tmp is not None:
                                nc.vector.tensor_scalar_max(
                                    hs[:, 0:BL], tmp[:, 0:BL], 0.0)
                                nc.vector.tensor_scalar_max(
                                    hs[:, n * BL:(n + 1) * BL],
                                    tmp[:, (n - 1) * BL: n * BL], 0.0)
                            else:
                                nc.vector.tensor_scalar_add(
                                    hs[:, 0:BL], hsl[:, 0:BL], 0.0)
                                nc.vector.tensor_scalar_add(
                                    hs[:, n * BL:(n + 1) * BL],
                                    hsl[:, (n - 1) * BL: n * BL], 0.0)
                            if n > 1:
                                nc.vector.scalar_tensor_tensor(
                                    out=hs[:, BL:n * BL],
                                    in0=tmp[:, 0:(n - 1) * BL],
                                    scalar=0.0, op0=ALU.max, op1=ALU.add,
                                    in1=hsl[:, BL:n * BL])
                        else:
                            # contracting: n2 = n-1; hs[s] = h[s] + h[s+1]
                            nc.vector.scalar_tensor_tensor(
                                out=hs[:],
                                in0=tmp[:, 0:(n - 1) * BL],
                                scalar=0.0, op0=ALU.max, op1=ALU.add,
                                in1=hsl[:, BL:n * BL])

                hs_prev = hs_next

                # interleave completed output chunks (lagged)
                for ch in range(YCH):
                    if not y_emitted[ch] and YREADY[ch] + YLAG <= d:
                        y_emitted[ch] = True
                        emit_y_chunk(ch)

            for ch in range(YCH):
                if not y_emitted[ch]:
                    y_emitted[ch] = True
                    emit_y_chunk(ch)

    nc.compile()
    return nc


_CACHE = {}


def _get_program():
    if "nc" not in _CACHE:
        _CACHE["nc"] = _build_program()
    return _CACHE["nc"]


def _host_indices():
    """Precompute gather indices for host-side pre/post permutation."""
    if "idx" in _CACHE:
        return _CACHE["idx"]
    # forward: xs[p, q] with q = CT*OFFB[d] + ct*(n*BL) + s*BL + b
    ct_of = np.empty(CT * TOT, dtype=np.int64)
    cell_of = np.empty(CT * TOT, dtype=np.int64)   # global cell index (d,s)
    b_of = np.empty(CT * TOT, dtype=np.int64)
    cell_base = 0
    for d in range(ND):
        n = N_D[d]
        q0 = CT * OFFB[d]
        blk = n * BL
        for ct in range(CT):
            qs = q0 + ct * blk
            idx = np.arange(blk)
            ct_of[qs:qs + blk] = ct
            cell_of[qs:qs + blk] = cell_base + idx // BL
            b_of[qs:qs + blk] = idx % BL
        cell_base += n
    # cell -> (i, j)
    ci = np.empty(H * W, dtype=np.int64)
    cj = np.empty(H * W, dtype=np.int64)
    cell_base = 0
    for d in range(ND):
        for s in range(N_D[d]):
            i = IMIN[d] + s
            ci[cell_base] = i
            cj[cell_base] = d - i
            cell_base += 1
    # inverse for y: qcell[i, j] = OFFB[d] + s*BL
    qcell = np.empty((H, W), dtype=np.int64)
    cell_base = 0
    for d in range(ND):
        for s in range(N_D[d]):
            i = IMIN[d] + s
            qcell[i, d - i] = OFFB[d] + s * BL
            cell_base += 1
    _CACHE["idx"] = (ct_of, cell_of, b_of, ci, cj, qcell)
    return _CACHE["idx"]


def kernel(input, weight_hh, weight_yh, bias):
    x = np.ascontiguousarray(np.asarray(input, dtype=np.float32))
    whh = np.asarray(weight_hh, dtype=np.float32)
    wyh = np.asarray(weight_yh, dtype=np.float32)
    b = np.asarray(bias, dtype=np.float32)

    nc = _get_program()
    ct_of, cell_of, b_of, ci, cj, qcell = _host_indices()

    whh16 = whh.astype(np.float16)
    wyh16 = wyh.astype(np.float16)
    biasp = np.ascontiguousarray(
        b.reshape(CT, P).T.astype(np.float32))       # [128, 4]

    # x gathered per core: [BL, C, H, W] -> arr3 [p, ct, cell, b]
    xg = x[:, :, ci, cj]                             # [B, C, 1024]
    in_maps = []
    for c in range(NCORES):
        arr = xg[c * BL:(c + 1) * BL]                # [BL, C, 1024]
        arr3 = arr.reshape(BL, CT, P, H * W).transpose(2, 1, 3, 0)
        xs_core = np.ascontiguousarray(
            arr3[np.arange(P)[:, None], ct_of[None, :], cell_of[None, :],
                 b_of[None, :]])
        in_maps.append({"xs": xs_core, "whh": whh16, "wyh": wyh16,
                        "biasp": biasp})

    res = bass_utils.run_bass_kernel_spmd(nc, in_maps,
                                          core_ids=list(range(NCORES)))

    out = np.empty((B, C, H, W), dtype=np.float32)
    qidx = qcell[None, :, :] + np.arange(BL)[:, None, None]   # [BL, H, W]
    for c in range(NCORES):
        ydev = res.results[c]["y"]                   # [512, 4096]
        out[c * BL:(c + 1) * BL] = ydev[:, qidx].transpose(1, 0, 2, 3)
    return out


# revision 5
# speedup vs baseline: 1.2532x; 1.2532x over previous
"""Trainium2 Bass kernel for the SE-sweep DAG-RNN (nn_DAG_RNN_se).

Reference semantics (B=32, C=512, H=W=32):
    h[i,j] = relu(x[:,:,i,j] + (h[i-1,j] + h[i,j-1]) @ W_hh)     # [B, C]
    y[i,j] = h[i,j] @ W_yh + bias

Strategy:
  * Data-parallel over batch: 8 cores x 4 batch elements, zero communication.
  * Anti-diagonal wavefront inside a core: diagonal d holds n_d cells; all
    cells of a diagonal are batched into one set of matmuls.
  * State layout is transposed: h^T [C(4x128 partitions), n_d*B_local] so
    W_hh chunks are the stationary matmul operand; N = 4*n_d <= 128.
  * The +x is folded into the PSUM accumulation via an identity matmul
    (start=True writes x, the 4 W-matmuls accumulate), so the scalar engine
    relu reads PSUM directly and the vector engine only builds the
    neighbour-sum h_sum via free-dim shifted adds (cells on a diagonal are
    ordered by row; neighbours on the next diagonal are adjacent slots).
  * fp16 state + weights (1 cyc/row on PE like bf16, ~8x finer mantissa);
    PSUM accumulates fp32.
  * Output transform y = h @ W_yh in [128,512]-wide matmuls per 512-column
    chunk of the hidden buffer, interleaved into PE bubbles as chunks
    complete.

The full (unsharded) numpy contract is `kernel(**inputs)`; the Bass program
is built and compiled once and cached at module level.
"""

import sys

if "/opt/trn_rl_repo" not in sys.path:
    sys.path.insert(0, "/opt/trn_rl_repo")

import numpy as np

import concourse.bass as bass
import concourse.mybir as mybir
import concourse.tile as tile
from concourse import bacc
from concourse import bass_utils

# ---------------------------------------------------------------- constants
B, C, H, W = 32, 512, 32, 32
NCORES = 8
BL = B // NCORES            # local batch per core = 4
ND = H + W - 1              # 63 diagonals
CT = 4                      # channel chunks of 128
P = 128
YCH = 8                     # output column chunks of 512

F32 = mybir.dt.float32
F16 = mybir.dt.float16
ALU = mybir.AluOpType
ACTF = mybir.ActivationFunctionType

N_D = [min(d, H - 1) - max(0, d - (W - 1)) + 1 for d in range(ND)]
IMIN = [max(0, d - (W - 1)) for d in range(ND)]
OFFB = [0] * (ND + 1)
for _d in range(ND):
    OFFB[_d + 1] = OFFB[_d] + N_D[_d] * BL
TOT = OFFB[ND]              # 4096 columns per chunk row

YREADY = [min(d for d in range(ND) if OFFB[d + 1] >= 512 * (ch + 1))
          for ch in range(YCH)]
YLAG = 2


def _build_program():
    nc = bacc.Bacc("TRN2", target_bir_lowering=False, debug=False,
                   num_devices=NCORES)

    xs = nc.dram_tensor("xs", [P, CT * TOT], F16, kind="ExternalInput").ap()
    whh = nc.dram_tensor("whh", [C, C], F16, kind="ExternalInput").ap()
    wyh = nc.dram_tensor("wyh", [C, C], F16, kind="ExternalInput").ap()
    ident = nc.dram_tensor("ident", [P, P], F16, kind="ExternalInput").ap()
    biasp = nc.dram_tensor("biasp", [P, CT], F32, kind="ExternalInput").ap()
    y = nc.dram_tensor("y", [C, TOT], F32, kind="ExternalOutput").ap()

    with tile.TileContext(nc) as tc:
        with (
            tc.tile_pool(name="persist", bufs=1) as persist,
            tc.tile_pool(name="hspool", bufs=3) as hspool,
            tc.tile_pool(name="ypool", bufs=4) as ypool,
            tc.tile_pool(name="recps", bufs=3, space="PSUM") as recps,
            tc.tile_pool(name="yps", bufs=2, space="PSUM") as yps,
        ):
            # ---- resident tensors ----
            whh_sb = persist.tile([P, CT * C], F16, name="whh_sb")
            wyh_sb = persist.tile([P, CT * C], F16, name="wyh_sb")
            id_sb = persist.tile([P, P], F16, name="id_sb")
            bias_sb = persist.tile([P, CT], F32, name="bias_sb")
            # hidden state, chunk-major: chunk k occupies cols [k*TOT,(k+1)*TOT)
            hj = persist.tile([P, CT * TOT], F16, name="hj")
            # full input, resident: col q = CT*OFFB[d] + ct*(n_d*BL) + s*BL + b
            xsb = persist.tile([P, CT * TOT], F16, name="xsb")

            for k in range(CT):
                nc.sync.dma_start(whh_sb[:, k * C:(k + 1) * C],
                                  whh[k * P:(k + 1) * P, :])
                nc.sync.dma_start(wyh_sb[:, k * C:(k + 1) * C],
                                  wyh[k * P:(k + 1) * P, :])
            nc.sync.dma_start(id_sb[:], ident[:])
            nc.sync.dma_start(bias_sb[:], biasp[:])
            NXD = 8
            for j in range(NXD):
                w = CT * TOT // NXD
                nc.sync.dma_start(xsb[:, j * w:(j + 1) * w],
                                  xs[:, j * w:(j + 1) * w])

            def w_slice(wsb, k, ct):
                return wsb[:, k * C + ct * P: k * C + ct * P + P]

            def hjs(k, c0, w):
                """h chunk-k cols [c0, c0+w) as an AP."""
                return hj[:, k * TOT + c0: k * TOT + c0 + w]

            def hj2(kbase, c0, w):
                """strided pair view: chunks kbase,kbase+1, cols [c0,c0+w)."""
                pair = hj[:, kbase * TOT:(kbase + 2) * TOT]
                return pair.rearrange("p (k q) -> p k q", k=2)[:, :, c0:c0 + w]

            y_emitted = [False] * YCH

            def emit_y_chunk(ch):
                for ct in range(CT):
                    psy = yps.tile([P, 512], F32, tag="psy",
                                   name=f"psy{ch}_{ct}")
                    for k in range(CT):
                        nc.tensor.matmul(
                            psy[:],
                            lhsT=w_slice(wyh_sb, k, ct),
                            rhs=hjs(k, ch * 512, 512),
                            start=(k == 0), stop=(k == CT - 1))
                    ysb = ypool.tile([P, 512], F32, tag="ysb",
                                     name=f"ysb{ch}_{ct}")
                    if (ch * CT + ct) % 2 == 0:
                        nc.vector.tensor_scalar_add(ysb[:], psy[:],
                                                    bias_sb[:, ct:ct + 1])
                    else:
                        nc.scalar.activation(ysb[:], psy[:], ACTF.Identity,
                                             bias=bias_sb[:, ct:ct + 1],
                                             scale=1.0)
                    nc.sync.dma_start(
                        y[ct * P:(ct + 1) * P, ch * 512:(ch + 1) * 512],
                        ysb[:])

            hs_prev = None     # list of 2 pair tiles [P, 2*N]
            for d in range(ND):
                n = N_D[d]
                N = n * BL
                x0 = CT * OFFB[d]

                if d + 1 < ND:
                    N2 = N_D[d + 1] * BL
                    hs_next = [hspool.tile([P, 2 * N2], F16, tag=f"hsp{pr}",
                                           name=f"hsp{pr}_{d + 1}")
                               for pr in range(2)]
                else:
                    hs_next = None

                if d == 0:
                    # h = relu(x): two pair-strided activations
                    for pr in range(2):
                        xv = xsb[:, x0 + 2 * pr * N: x0 + (2 * pr + 2) * N]
                        xv = xv.rearrange("p (k q) -> p k q", k=2)
                        nc.scalar.activation(hj2(2 * pr, OFFB[d], N), xv,
                                             ACTF.Relu)
                else:
                    ps_pair = [recps.tile([P, 1024], F32, tag="ps",
                                          name=f"ps{d}_{pr}")
                               for pr in range(2)]
                    for g in range(CT):
                        pr, gl = divmod(g, 2)
                        out = ps_pair[pr][:, gl * 512: gl * 512 + N]
                        nc.tensor.matmul(out, lhsT=id_sb[:],
                                         rhs=xsb[:, x0 + g * N:
                                                 x0 + (g + 1) * N],
                                         start=True, stop=False)
                        korder = [(g + 1) % CT, (g + 2) % CT,
                                  (g + 3) % CT, g]
                        for idx, k in enumerate(korder):
                            kp, kl = divmod(k, 2)
                            nc.tensor.matmul(
                                out,
                                lhsT=w_slice(whh_sb, k, g),
                                rhs=hs_prev[kp][:, kl * N:(kl + 1) * N],
                                start=False, stop=(idx == CT - 1))
                        if g % 2 == 1:
                            # relu for the completed pair, PSUM -> h (fp16)
                            pv = ps_pair[pr].tensor.ap().rearrange(
                                "p (k q) -> p k q", k=2)[:, :, 0:N]
                            nc.scalar.activation(hj2(2 * pr, OFFB[d], N),
                                                 pv, ACTF.Relu)

                # h_sum for diag d+1 from h (pair-strided shifted adds)
                if hs_next is not None:
                    for pr in range(2):
                        hs = hs_next[pr]
                        hsv = hs.rearrange("p (k q) -> p k q", k=2)
                        if d + 1 <= W - 1:
                            # expanding: n2 = n+1
                            nc.vector.tensor_scalar_add(
                                hsv[:, :, 0:BL],
                                hj2(2 * pr, OFFB[d], BL), 0.0)
                            nc.vector.tensor_scalar_add(
                                hsv[:, :, n * BL:(n + 1) * BL],
                                hj2(2 * pr, OFFB[d] + (n - 1) * BL, BL), 0.0)
                            if n > 1:
                                nc.vector.scalar_tensor_tensor(
                                    out=hsv[:, :, BL:n * BL],
                                    in0=hj2(2 * pr, OFFB[d], (n - 1) * BL),
                                    scalar=0.0, op0=ALU.bypass, op1=ALU.add,
                                    in1=hj2(2 * pr, OFFB[d] + BL,
                                            (n - 1) * BL))
                        else:
                            # contracting: n2 = n-1; hs[s] = h[s] + h[s+1]
                            nc.vector.scalar_tensor_tensor(
                                out=hsv[:, :, 0:(n - 1) * BL],
                                in0=hj2(2 * pr, OFFB[d], (n - 1) * BL),
                                scalar=0.0, op0=ALU.bypass, op1=ALU.add,
                                in1=hj2(2 * pr, OFFB[d] + BL, (n - 1) * BL))

                hs_prev = hs_next

                for ch in range(YCH):
                    if not y_emitted[ch] and YREADY[ch] + YLAG <= d:
                        y_emitted[ch] = True
                        emit_y_chunk(ch)

            for ch in range(YCH):
                if not y_emitted[ch]:
                    y_emitted[ch] = True
                    emit_y_chunk(ch)

    nc.compile()
    return nc


_CACHE = {}


def _get_program():
    if "nc" not in _CACHE:
        _CACHE["nc"] = _build_program()
    return _CACHE["nc"]


def _host_indices():
    """Precompute gather indices for host-side pre/post permutation."""
    if "idx" in _CACHE:
        return _CACHE["idx"]
    ct_of = np.empty(CT * TOT, dtype=np.int64)
    cell_of = np.empty(CT * TOT, dtype=np.int64)
    b_of = np.empty(CT * TOT, dtype=np.int64)
    cell_base = 0
    for d in range(ND):
        n = N_D[d]
        q0 = CT * OFFB[d]
        blk = n * BL
        for ct in range(CT):
            qs = q0 + ct * blk
            idx = np.arange(blk)
            ct_of[qs:qs + blk] = ct
            cell_of[qs:qs + blk] = cell_base + idx // BL
            b_of[qs:qs + blk] = idx % BL
        cell_base += n
    ci = np.empty(H * W, dtype=np.int64)
    cj = np.empty(H * W, dtype=np.int64)
    qcell = np.empty((H, W), dtype=np.int64)
    cell_base = 0
    for d in range(ND):
        for s in range(N_D[d]):
            i = IMIN[d] + s
            ci[cell_base] = i
            cj[cell_base] = d - i
            qcell[i, d - i] = OFFB[d] + s * BL
            cell_base += 1
    _CACHE["idx"] = (ct_of, cell_of, b_of, ci, cj, qcell)
    return _CACHE["idx"]


def make_in_maps(x, whh, wyh, b):
    ct_of, cell_of, b_of, ci, cj, qcell = _host_indices()
    whh16 = whh.astype(np.float16)
    wyh16 = wyh.astype(np.float16)
    id16 = np.eye(P, dtype=np.float16)
    biasp = np.ascontiguousarray(b.reshape(CT, P).T.astype(np.float32))
    xg = x[:, :, ci, cj]                             # [B, C, 1024]
    in_maps = []
    for c in range(NCORES):
        arr = xg[c * BL:(c + 1) * BL]                # [BL, C, 1024]
        arr3 = arr.reshape(BL, CT, P, H * W).transpose(2, 1, 3, 0)
        xs_core = np.ascontiguousarray(
            arr3[np.arange(P)[:, None], ct_of[None, :], cell_of[None, :],
                 b_of[None, :]].astype(np.float16))
        in_maps.append({"xs": xs_core, "whh": whh16, "wyh": wyh16,
                        "ident": id16, "biasp": biasp})
    return in_maps


def kernel(input, weight_hh, weight_yh, bias):
    x = np.ascontiguousarray(np.asarray(input, dtype=np.float32))
    whh = np.asarray(weight_hh, dtype=np.float32)
    wyh = np.asarray(weight_yh, dtype=np.float32)
    b = np.asarray(bias, dtype=np.float32)

    nc = _get_program()
    in_maps = make_in_maps(x, whh, wyh, b)
    res = bass_utils.run_bass_kernel_spmd(nc, in_maps,
                                          core_ids=list(range(NCORES)))

    _, _, _, _, _, qcell = _host_indices()
    out = np.empty((B, C, H, W), dtype=np.float32)
    qidx = qcell[None, :, :] + np.arange(BL)[:, None, None]
    for c in range(NCORES):
        ydev = res.results[c]["y"]                   # [512, 4096]
        out[c * BL:(c + 1) * BL] = ydev[:, qidx].transpose(1, 0, 2, 3)
    return out


# revision 7
# speedup vs baseline: 1.2765x; 1.0186x over previous
"""Trainium2 Bass kernel for the SE-sweep DAG-RNN (nn_DAG_RNN_se).

Reference semantics (B=32, C=512, H=W=32):
    h[i,j] = relu(x[:,:,i,j] + (h[i-1,j] + h[i,j-1]) @ W_hh)     # [B, C]
    y[i,j] = h[i,j] @ W_yh + bias

Strategy:
  * Data-parallel over batch: 8 cores x 4 batch elements, zero communication.
  * Anti-diagonal wavefront inside a core: diagonal d holds n_d cells; all
    cells of a diagonal are batched into one set of matmuls.
  * State layout is transposed: h^T [C(4x128 partitions), n_d*B_local] so
    W_hh chunks are the stationary matmul operand; N = 4*n_d <= 128.
  * The +x is folded into the PSUM accumulation via an identity matmul
    (start=True writes x, the 4 W-matmuls accumulate), so the scalar engine
    relu reads PSUM directly and the vector engine only builds the
    neighbour-sum h_sum via free-dim shifted adds (cells on a diagonal are
    ordered by row; neighbours on the next diagonal are adjacent slots).
  * fp16 state + weights (1 cyc/row on PE like bf16, ~8x finer mantissa);
    PSUM accumulates fp32.
  * Output transform y = h @ W_yh in [128,512]-wide matmuls per 512-column
    chunk of the hidden buffer, interleaved into PE bubbles as chunks
    complete.

The full (unsharded) numpy contract is `kernel(**inputs)`; the Bass program
is built and compiled once and cached at module level.
"""

import sys

if "/opt/trn_rl_repo" not in sys.path:
    sys.path.insert(0, "/opt/trn_rl_repo")

import numpy as np

import concourse.bass as bass
import concourse.mybir as mybir
import concourse.tile as tile
from concourse import bacc
from concourse import bass_utils

# ---------------------------------------------------------------- constants
B, C, H, W = 32, 512, 32, 32
NCORES = 8
BL = B // NCORES            # local batch per core = 4
ND = H + W - 1              # 63 diagonals
CT = 4                      # channel chunks of 128
P = 128
YCH = 8                     # output column chunks of 512

F32 = mybir.dt.float32
F16 = mybir.dt.float16
ALU = mybir.AluOpType
ACTF = mybir.ActivationFunctionType

N_D = [min(d, H - 1) - max(0, d - (W - 1)) + 1 for d in range(ND)]
IMIN = [max(0, d - (W - 1)) for d in range(ND)]
OFFB = [0] * (ND + 1)
for _d in range(ND):
    OFFB[_d + 1] = OFFB[_d] + N_D[_d] * BL
TOT = OFFB[ND]              # 4096 columns per chunk row

YREADY = [min(d for d in range(ND) if OFFB[d + 1] >= 512 * (ch + 1))
          for ch in range(YCH)]
YLAG = 2


def _build_program():
    nc = bacc.Bacc("TRN2", target_bir_lowering=False, debug=False,
                   num_devices=NCORES)

    xs = nc.dram_tensor("xs", [P, CT * TOT], F16, kind="ExternalInput").ap()
    whh = nc.dram_tensor("whh", [C, C], F16, kind="ExternalInput").ap()
    wyh = nc.dram_tensor("wyh", [C, C], F16, kind="ExternalInput").ap()
    ident = nc.dram_tensor("ident", [P, P], F16, kind="ExternalInput").ap()
    biasp = nc.dram_tensor("biasp", [P, CT], F32, kind="ExternalInput").ap()
    y = nc.dram_tensor("y", [C, TOT], F32, kind="ExternalOutput").ap()

    with tile.TileContext(nc) as tc:
        with (
            tc.tile_pool(name="persist", bufs=1) as persist,
            tc.tile_pool(name="hspool", bufs=4) as hspool,
            tc.tile_pool(name="ypool", bufs=4) as ypool,
            tc.tile_pool(name="recps", bufs=3, space="PSUM") as recps,
            tc.tile_pool(name="yps", bufs=2, space="PSUM") as yps,
        ):
            # ---- resident tensors ----
            whh_sb = persist.tile([P, CT * C], F16, name="whh_sb")
            wyh_sb = persist.tile([P, CT * C], F16, name="wyh_sb")
            id_sb = persist.tile([P, P], F16, name="id_sb")
            bias_sb = persist.tile([P, CT], F32, name="bias_sb")
            # hidden state, chunk-major: chunk k occupies cols [k*TOT,(k+1)*TOT)
            hj = persist.tile([P, CT * TOT], F16, name="hj")
            # full input, resident: col q = CT*OFFB[d] + ct*(n_d*BL) + s*BL + b
            xsb = persist.tile([P, CT * TOT], F16, name="xsb")

            for k in range(CT):
                nc.sync.dma_start(whh_sb[:, k * C:(k + 1) * C],
                                  whh[k * P:(k + 1) * P, :])
                nc.sync.dma_start(wyh_sb[:, k * C:(k + 1) * C],
                                  wyh[k * P:(k + 1) * P, :])
            nc.sync.dma_start(id_sb[:], ident[:])
            nc.sync.dma_start(bias_sb[:], biasp[:])
            NXD = 8
            for j in range(NXD):
                w = CT * TOT // NXD
                nc.sync.dma_start(xsb[:, j * w:(j + 1) * w],
                                  xs[:, j * w:(j + 1) * w])

            def w_slice(wsb, k, ct):
                return wsb[:, k * C + ct * P: k * C + ct * P + P]

            def hjs(k, c0, w):
                """h chunk-k cols [c0, c0+w) as an AP."""
                return hj[:, k * TOT + c0: k * TOT + c0 + w]

            def hj2(kbase, c0, w):
                """strided pair view: chunks kbase,kbase+1, cols [c0,c0+w)."""
                pair = hj[:, kbase * TOT:(kbase + 2) * TOT]
                return pair.rearrange("p (k q) -> p k q", k=2)[:, :, c0:c0 + w]

            y_emitted = [False] * YCH

            def emit_y_chunk(ch):
                for ct in range(CT):
                    psy = yps.tile([P, 512], F32, tag="psy",
                                   name=f"psy{ch}_{ct}")
                    for k in range(CT):
                        nc.tensor.matmul(
                            psy[:],
                            lhsT=w_slice(wyh_sb, k, ct),
                            rhs=hjs(k, ch * 512, 512),
                            start=(k == 0), stop=(k == CT - 1))
                    ysb = ypool.tile([P, 512], F32, tag="ysb",
                                     name=f"ysb{ch}_{ct}")
                    if (ch * CT + ct) % 2 == 0:
                        nc.vector.tensor_scalar_add(ysb[:], psy[:],
                                                    bias_sb[:, ct:ct + 1])
                    else:
                        nc.scalar.activation(ysb[:], psy[:], ACTF.Identity,
                                             bias=bias_sb[:, ct:ct + 1],
                                             scale=1.0)
                    nc.sync.dma_start(
                        y[ct * P:(ct + 1) * P, ch * 512:(ch + 1) * 512],
                        ysb[:])

            hs_prev = None     # list of 2 pair tiles [P, 2*N]
            for d in range(ND):
                n = N_D[d]
                N = n * BL
                x0 = CT * OFFB[d]

                if d + 1 < ND:
                    N2 = N_D[d + 1] * BL
                    hs_next = [hspool.tile([P, 2 * N2], F16, tag=f"hsp{pr}",
                                           name=f"hsp{pr}_{d + 1}")
                               for pr in range(2)]
                else:
                    hs_next = None

                if d == 0:
                    # h = relu(x): two pair-strided activations
                    for pr in range(2):
                        xv = xsb[:, x0 + 2 * pr * N: x0 + (2 * pr + 2) * N]
                        xv = xv.rearrange("p (k q) -> p k q", k=2)
                        nc.scalar.activation(hj2(2 * pr, OFFB[d], N), xv,
                                             ACTF.Relu)
                else:
                    ps_pair = [recps.tile([P, 1024], F32, tag="ps",
                                          name=f"ps{d}_{pr}")
                               for pr in range(2)]

                    def g_out(g):
                        pr, gl = divmod(g, 2)
                        return ps_pair[pr][:, gl * 512: gl * 512 + N]

                    NP = N  # capture for lambdas below
                    # early sub-burst: x (identity) + pair-A h_sum chunks.
                    # pair-B chunks of the previous diagonal finish ~1us
                    # later, so deferring their matmuls decouples the PE
                    # stream from the h_sum production chain.
                    for g in range(CT):
                        nc.tensor.matmul(g_out(g), lhsT=id_sb[:],
                                         rhs=xsb[:, x0 + g * NP:
                                                 x0 + (g + 1) * NP],
                                         start=True, stop=False)
                        for k in (0, 1) if g % 2 == 0 else (1, 0):
                            nc.tensor.matmul(
                                g_out(g),
                                lhsT=w_slice(whh_sb, k, g),
                                rhs=hs_prev[0][:, k * NP:(k + 1) * NP],
                                start=False, stop=False)
                    # late sub-burst: pair-B chunks, stop on the last
                    for g in range(CT):
                        ks = (2, 3) if g % 2 == 0 else (3, 2)
                        for idx, k in enumerate(ks):
                            nc.tensor.matmul(
                                g_out(g),
                                lhsT=w_slice(whh_sb, k, g),
                                rhs=hs_prev[1][:, (k - 2) * NP:
                                               (k - 1) * NP],
                                start=False, stop=(idx == 1))
                        if g % 2 == 1:
                            pr = g // 2
                            # relu for the completed pair, PSUM -> h (fp16)
                            pv = ps_pair[pr].tensor.ap().rearrange(
                                "p (k q) -> p k q", k=2)[:, :, 0:N]
                            nc.scalar.activation(hj2(2 * pr, OFFB[d], N),
                                                 pv, ACTF.Relu)

                # h_sum for diag d+1 from h (pair-strided shifted adds)
                if hs_next is not None:
                    for pr in range(2):
                        hs = hs_next[pr]
                        hsv = hs.rearrange("p (k q) -> p k q", k=2)
                        if d + 1 <= W - 1:
                            # expanding: n2 = n+1
                            nc.vector.tensor_scalar_add(
                                hsv[:, :, 0:BL],
                                hj2(2 * pr, OFFB[d], BL), 0.0)
                            nc.vector.tensor_scalar_add(
                                hsv[:, :, n * BL:(n + 1) * BL],
                                hj2(2 * pr, OFFB[d] + (n - 1) * BL, BL), 0.0)
                            if n > 1:
                                nc.vector.scalar_tensor_tensor(
                                    out=hsv[:, :, BL:n * BL],
                                    in0=hj2(2 * pr, OFFB[d], (n - 1) * BL),
                                    scalar=0.0, op0=ALU.bypass, op1=ALU.add,
                                    in1=hj2(2 * pr, OFFB[d] + BL,
                                            (n - 1) * BL))
                        else:
                            # contracting: n2 = n-1; hs[s] = h[s] + h[s+1]
                            nc.vector.scalar_tensor_tensor(
                                out=hsv[:, :, 0:(n - 1) * BL],
                                in0=hj2(2 * pr, OFFB[d], (n - 1) * BL),
                                scalar=0.0, op0=ALU.bypass, op1=ALU.add,
                                in1=hj2(2 * pr, OFFB[d] + BL, (n - 1) * BL))

                hs_prev = hs_next

                for ch in range(YCH):
                    if not y_emitted[ch] and YREADY[ch] + YLAG <= d:
                        y_emitted[ch] = True
                        emit_y_chunk(ch)

            for ch in range(YCH):
                if not y_emitted[ch]:
                    y_emitted[ch] = True
                    emit_y_chunk(ch)

    nc.compile()
    return nc


_CACHE = {}


def _get_program():
    if "nc" not in _CACHE:
        _CACHE["nc"] = _build_program()
    return _CACHE["nc"]


def _host_indices():
    """Precompute gather indices for host-side pre/post permutation."""
    if "idx" in _CACHE:
        return _CACHE["idx"]
    ct_of = np.empty(CT * TOT, dtype=np.int64)
    cell_of = np.empty(CT * TOT, dtype=np.int64)
    b_of = np.empty(CT * TOT, dtype=np.int64)
    cell_base = 0
    for d in range(ND):
        n = N_D[d]
        q0 = CT * OFFB[d]
        blk = n * BL
        for ct in range(CT):
            qs = q0 + ct * blk
            idx = np.arange(blk)
            ct_of[qs:qs + blk] = ct
            cell_of[qs:qs + blk] = cell_base + idx // BL
            b_of[qs:qs + blk] = idx % BL
        cell_base += n
    ci = np.empty(H * W, dtype=np.int64)
    cj = np.empty(H * W, dtype=np.int64)
    qcell = np.empty((H, W), dtype=np.int64)
    cell_base = 0
    for d in range(ND):
        for s in range(N_D[d]):
            i = IMIN[d] + s
            ci[cell_base] = i
            cj[cell_base] = d - i
            qcell[i, d - i] = OFFB[d] + s * BL
            cell_base += 1
    _CACHE["idx"] = (ct_of, cell_of, b_of, ci, cj, qcell)
    return _CACHE["idx"]


def make_in_maps(x, whh, wyh, b):
    ct_of, cell_of, b_of, ci, cj, qcell = _host_indices()
    whh16 = whh.astype(np.float16)
    wyh16 = wyh.astype(np.float16)
    id16 = np.eye(P, dtype=np.float16)
    biasp = np.ascontiguousarray(b.reshape(CT, P).T.astype(np.float32))
    xg = x[:, :, ci, cj]                             # [B, C, 1024]
    in_maps = []
    for c in range(NCORES):
        arr = xg[c * BL:(c + 1) * BL]                # [BL, C, 1024]
        arr3 = arr.reshape(BL, CT, P, H * W).transpose(2, 1, 3, 0)
        xs_core = np.ascontiguousarray(
            arr3[np.arange(P)[:, None], ct_of[None, :], cell_of[None, :],
                 b_of[None, :]].astype(np.float16))
        in_maps.append({"xs": xs_core, "whh": whh16, "wyh": wyh16,
                        "ident": id16, "biasp": biasp})
    return in_maps


def kernel(input, weight_hh, weight_yh, bias):
    x = np.ascontiguousarray(np.asarray(input, dtype=np.float32))
    whh = np.asarray(weight_hh, dtype=np.float32)
    wyh = np.asarray(weight_yh, dtype=np.float32)
    b = np.asarray(bias, dtype=np.float32)

    nc = _get_program()
    in_maps = make_in_maps(x, whh, wyh, b)
    res = bass_utils.run_bass_kernel_spmd(nc, in_maps,
                                          core_ids=list(range(NCORES)))

    _, _, _, _, _, qcell = _host_indices()
    out = np.empty((B, C, H, W), dtype=np.float32)
    qidx = qcell[None, :, :] + np.arange(BL)[:, None, None]
    for c in range(NCORES):
        ydev = res.results[c]["y"]                   # [512, 4096]
        out[c * BL:(c + 1) * BL] = ydev[:, qidx].transpose(1, 0, 2, 3)
    return out


# revision 13
# speedup vs baseline: 1.3181x; 1.0325x over previous
"""Trainium2 Bass kernel for the SE-sweep DAG-RNN (nn_DAG_RNN_se).

Reference semantics (B=32, C=512, H=W=32):
    h[i,j] = relu(x[:,:,i,j] + (h[i-1,j] + h[i,j-1]) @ W_hh)     # [B, C]
    y[i,j] = h[i,j] @ W_yh + bias

Strategy:
  * Data-parallel over batch: 8 cores x 4 batch elements, zero communication.
  * Anti-diagonal wavefront inside a core: diagonal d holds n_d cells; all
    cells of a diagonal are batched into one set of matmuls.
  * State layout is transposed: h^T [C(4x128 partitions), n_d*B_local] so
    W_hh chunks are the stationary matmul operand; N = 4*n_d <= 128.
  * The +x is folded into the PSUM accumulation via an identity matmul
    (start=True writes x, the 4 W-matmuls accumulate), so the scalar engine
    relu reads PSUM directly and the vector engine only builds the
    neighbour-sum h_sum via free-dim shifted adds (cells on a diagonal are
    ordered by row; neighbours on the next diagonal are adjacent slots).
  * fp16 state + weights (1 cyc/row on PE like bf16, ~8x finer mantissa);
    PSUM accumulates fp32.
  * Output transform y = h @ W_yh in [128,512]-wide matmuls per 512-column
    chunk of the hidden buffer, interleaved into PE bubbles as chunks
    complete.

The full (unsharded) numpy contract is `kernel(**inputs)`; the Bass program
is built and compiled once and cached at module level.
"""

import sys

if "/opt/trn_rl_repo" not in sys.path:
    sys.path.insert(0, "/opt/trn_rl_repo")

import numpy as np

import concourse.bass as bass
import concourse.mybir as mybir
import concourse.tile as tile
from concourse import bacc
from concourse import bass_utils

# ---------------------------------------------------------------- constants
B, C, H, W = 32, 512, 32, 32
NCORES = 8
BL = B // NCORES            # local batch per core = 4
ND = H + W - 1              # 63 diagonals
CT = 4                      # channel chunks of 128
P = 128
YCH = 8                     # output column chunks of 512

F32 = mybir.dt.float32
F16 = mybir.dt.float16
ALU = mybir.AluOpType
ACTF = mybir.ActivationFunctionType

N_D = [min(d, H - 1) - max(0, d - (W - 1)) + 1 for d in range(ND)]
IMIN = [max(0, d - (W - 1)) for d in range(ND)]
OFFB = [0] * (ND + 1)
for _d in range(ND):
    OFFB[_d + 1] = OFFB[_d] + N_D[_d] * BL
TOT = OFFB[ND]              # 4096 columns per chunk row

# y output chunks (col0, width): 512-wide, last one split so the forced
# serial tail after the final diagonal is half as long
YCHUNKS = [(i * 512, 512) for i in range(YCH - 1)] + [(3584, 256), (3840, 256)]
YREADY = [min(d for d in range(ND) if OFFB[d + 1] >= c0 + w)
          for (c0, w) in YCHUNKS]
YLAG = 2


def _build_program():
    nc = bacc.Bacc("TRN2", target_bir_lowering=False, debug=False,
                   num_devices=NCORES)

    xs = nc.dram_tensor("xs", [P, CT * TOT], F16, kind="ExternalInput").ap()
    whh = nc.dram_tensor("whh", [C, C], F16, kind="ExternalInput").ap()
    wyh = nc.dram_tensor("wyh", [C, C], F16, kind="ExternalInput").ap()
    ident = nc.dram_tensor("ident", [P, P], F16, kind="ExternalInput").ap()
    biasp = nc.dram_tensor("biasp", [P, CT], F32, kind="ExternalInput").ap()
    y = nc.dram_tensor("y", [C, TOT], F32, kind="ExternalOutput").ap()

    with tile.TileContext(nc) as tc:
        with (
            tc.tile_pool(name="persist", bufs=1) as persist,
            tc.tile_pool(name="hspool", bufs=4) as hspool,
            tc.tile_pool(name="ypool", bufs=4) as ypool,
            tc.tile_pool(name="recps", bufs=6, space="PSUM") as recps,
            tc.tile_pool(name="yps", bufs=2, space="PSUM") as yps,
        ):
            # ---- resident tensors ----
            whh_sb = persist.tile([P, CT * C], F16, name="whh_sb")
            wyh_sb = persist.tile([P, CT * C], F16, name="wyh_sb")
            id_sb = persist.tile([P, P], F16, name="id_sb")
            bias_sb = persist.tile([P, CT], F32, name="bias_sb")
            # hidden state, chunk-major: chunk k occupies cols [k*TOT,(k+1)*TOT)
            hj = persist.tile([P, CT * TOT], F16, name="hj")
            # full input, resident: col q = CT*OFFB[d] + ct*(n_d*BL) + s*BL + b
            xsb = persist.tile([P, CT * TOT], F16, name="xsb")

            # Startup ordering matters: the first diagonals need (in order)
            # a small x prefix, the identity, and W_hh. W_yh/bias are not
            # needed until the first y chunk (~diag 18). Spread across the
            # sync (HWDGE) and gpsimd (SWDGE) queues for parallelism.
            nc.sync.dma_start(xsb[:, 0:1024], xs[:, 0:1024])
            nc.sync.dma_start(id_sb[:], ident[:])
            for k in range(CT):
                nc.sync.dma_start(whh_sb[:, k * C:(k + 1) * C],
                                  whh[k * P:(k + 1) * P, :])
            NXD = 6
            w = (CT * TOT - 1024) // NXD
            for j in range(NXD):
                c0 = 1024 + j * w
                c1 = CT * TOT if j == NXD - 1 else c0 + w
                eng = nc.sync if j % 2 == 0 else nc.gpsimd
                eng.dma_start(xsb[:, c0:c1], xs[:, c0:c1])
            for k in range(CT):
                nc.gpsimd.dma_start(wyh_sb[:, k * C:(k + 1) * C],
                                    wyh[k * P:(k + 1) * P, :])
            nc.gpsimd.dma_start(bias_sb[:], biasp[:])

            def w_slice(wsb, k, ct):
                return wsb[:, k * C + ct * P: k * C + ct * P + P]

            def hjs(k, c0, w):
                """h chunk-k cols [c0, c0+w) as an AP."""
                return hj[:, k * TOT + c0: k * TOT + c0 + w]

            def hj2(kbase, c0, w):
                """strided pair view: chunks kbase,kbase+1, cols [c0,c0+w)."""
                pair = hj[:, kbase * TOT:(kbase + 2) * TOT]
                return pair.rearrange("p (k q) -> p k q", k=2)[:, :, c0:c0 + w]

            y_emitted = [False] * len(YCHUNKS)

            def emit_y_chunk(ch):
                c0, wd = YCHUNKS[ch]
                for ct in range(CT):
                    psy = yps.tile([P, 512], F32, tag="psy",
                                   name=f"psy{ch}_{ct}")
                    for k in range(CT):
                        nc.tensor.matmul(
                            psy[:, 0:wd],
                            lhsT=w_slice(wyh_sb, k, ct),
                            rhs=hjs(k, c0, wd),
                            start=(k == 0), stop=(k == CT - 1))
                    ysb = ypool.tile([P, 512], F32, tag="ysb",
                                     name=f"ysb{ch}_{ct}")
                    nc.scalar.activation(ysb[:, 0:wd], psy[:, 0:wd],
                                         ACTF.Identity,
                                         bias=bias_sb[:, ct:ct + 1],
                                         scale=1.0)
                    nc.sync.dma_start(y[ct * P:(ct + 1) * P, c0:c0 + wd],
                                      ysb[:, 0:wd])

            hs_prev = None     # list of 2 pair tiles [P, 2*N]
            for d in range(ND):
                n = N_D[d]
                N = n * BL
                x0 = CT * OFFB[d]

                if d + 1 < ND:
                    N2 = N_D[d + 1] * BL
                    hs_next = [hspool.tile([P, 2 * N2], F16, tag=f"hsp{pr}",
                                           name=f"hsp{pr}_{d + 1}")
                               for pr in range(2)]
                else:
                    hs_next = None

                if d == 0:
                    # h = relu(x): two pair-strided activations
                    for pr in range(2):
                        xv = xsb[:, x0 + 2 * pr * N: x0 + (2 * pr + 2) * N]
                        xv = xv.rearrange("p (k q) -> p k q", k=2)
                        nc.scalar.activation(hj2(2 * pr, OFFB[d], N), xv,
                                             ACTF.Relu)
                else:
                    psg = [recps.tile([P, 512], F32, tag="ps",
                                      name=f"ps{d}_{g}")
                           for g in range(CT)]
                    # early sub-burst: x (identity) + pair-A h_sum chunks.
                    # pair-B chunks of the previous diagonal finish ~1us
                    # later, so deferring their matmuls decouples the PE
                    # stream from the h_sum production chain.
                    for g in range(CT):
                        nc.tensor.matmul(psg[g][:, 0:N], lhsT=id_sb[:],
                                         rhs=xsb[:, x0 + g * N:
                                                 x0 + (g + 1) * N],
                                         start=True, stop=False)
                        for k in (0, 1) if g % 2 == 0 else (1, 0):
                            nc.tensor.matmul(
                                psg[g][:, 0:N],
                                lhsT=w_slice(whh_sb, k, g),
                                rhs=hs_prev[0][:, k * N:(k + 1) * N],
                                start=False, stop=False)
                    # late sub-burst: pair-B chunks, stop on the last;
                    # relu per chunk right after its stop, alternating
                    # engines so the two relus of a pair run concurrently
                    for g in range(CT):
                        ks = (2, 3) if g % 2 == 0 else (3, 2)
                        for idx, k in enumerate(ks):
                            nc.tensor.matmul(
                                psg[g][:, 0:N],
                                lhsT=w_slice(whh_sb, k, g),
                                rhs=hs_prev[1][:, (k - 2) * N:
                                               (k - 1) * N],
                                start=False, stop=(idx == 1))
                        if g % 2 == 0:
                            nc.vector.tensor_scalar_max(
                                hjs(g, OFFB[d], N), psg[g][:, 0:N], 0.0)
                        else:
                            nc.scalar.activation(hjs(g, OFFB[d], N),
                                                 psg[g][:, 0:N], ACTF.Relu)

                # h_sum for diag d+1 from h (pair-strided shifted adds)
                if hs_next is not None:
                    for pr in range(2):
                        hs = hs_next[pr]
                        hsv = hs.rearrange("p (k q) -> p k q", k=2)
                        if d + 1 <= W - 1:
                            # expanding: n2 = n+1
                            nc.vector.tensor_scalar_add(
                                hsv[:, :, 0:BL],
                                hj2(2 * pr, OFFB[d], BL), 0.0)
                            nc.vector.tensor_scalar_add(
                                hsv[:, :, n * BL:(n + 1) * BL],
                                hj2(2 * pr, OFFB[d] + (n - 1) * BL, BL), 0.0)
                            if n > 1:
                                nc.vector.scalar_tensor_tensor(
                                    out=hsv[:, :, BL:n * BL],
                                    in0=hj2(2 * pr, OFFB[d], (n - 1) * BL),
                                    scalar=0.0, op0=ALU.bypass, op1=ALU.add,
                                    in1=hj2(2 * pr, OFFB[d] + BL,
                                            (n - 1) * BL))
                        else:
                            # contracting: n2 = n-1; hs[s] = h[s] + h[s+1]
                            nc.vector.scalar_tensor_tensor(
                                out=hsv[:, :, 0:(n - 1) * BL],
                                in0=hj2(2 * pr, OFFB[d], (n - 1) * BL),
                                scalar=0.0, op0=ALU.bypass, op1=ALU.add,
                                in1=hj2(2 * pr, OFFB[d] + BL, (n - 1) * BL))

                hs_prev = hs_next

                for ch in range(len(YCHUNKS)):
                    if not y_emitted[ch] and YREADY[ch] + YLAG <= d:
                        y_emitted[ch] = True
                        emit_y_chunk(ch)

            for ch in range(len(YCHUNKS)):
                if not y_emitted[ch]:
                    y_emitted[ch] = True
                    emit_y_chunk(ch)

    nc.compile()
    return nc


_CACHE = {}


def _get_program():
    if "nc" not in _CACHE:
        _CACHE["nc"] = _build_program()
    return _CACHE["nc"]


def _host_indices():
    """Precompute gather indices for host-side pre/post permutation."""
    if "idx" in _CACHE:
        return _CACHE["idx"]
    ct_of = np.empty(CT * TOT, dtype=np.int64)
    cell_of = np.empty(CT * TOT, dtype=np.int64)
    b_of = np.empty(CT * TOT, dtype=np.int64)
    cell_base = 0
    for d in range(ND):
        n = N_D[d]
        q0 = CT * OFFB[d]
        blk = n * BL
        for ct in range(CT):
            qs = q0 + ct * blk
            idx = np.arange(blk)
            ct_of[qs:qs + blk] = ct
            cell_of[qs:qs + blk] = cell_base + idx // BL
            b_of[qs:qs + blk] = idx % BL
        cell_base += n
    ci = np.empty(H * W, dtype=np.int64)
    cj = np.empty(H * W, dtype=np.int64)
    qcell = np.empty((H, W), dtype=np.int64)
    cell_base = 0
    for d in range(ND):
        for s in range(N_D[d]):
            i = IMIN[d] + s
            ci[cell_base] = i
            cj[cell_base] = d - i
            qcell[i, d - i] = OFFB[d] + s * BL
            cell_base += 1
    _CACHE["idx"] = (ct_of, cell_of, b_of, ci, cj, qcell)
    return _CACHE["idx"]


def make_in_maps(x, whh, wyh, b):
    ct_of, cell_of, b_of, ci, cj, qcell = _host_indices()
    whh16 = whh.astype(np.float16)
    wyh16 = wyh.astype(np.float16)
    id16 = np.eye(P, dtype=np.float16)
    biasp = np.ascontiguousarray(b.reshape(CT, P).T.astype(np.float32))
    xg = x[:, :, ci, cj]                             # [B, C, 1024]
    in_maps = []
    for c in range(NCORES):
        arr = xg[c * BL:(c + 1) * BL]                # [BL, C, 1024]
        arr3 = arr.reshape(BL, CT, P, H * W).transpose(2, 1, 3, 0)
        xs_core = np.ascontiguousarray(
            arr3[np.arange(P)[:, None], ct_of[None, :], cell_of[None, :],
                 b_of[None, :]].astype(np.float16))
        in_maps.append({"xs": xs_core, "whh": whh16, "wyh": wyh16,
                        "ident": id16, "biasp": biasp})
    return in_maps


def kernel(input, weight_hh, weight_yh, bias):
    x = np.ascontiguousarray(np.asarray(input, dtype=np.float32))
    whh = np.asarray(weight_hh, dtype=np.float32)
    wyh = np.asarray(weight_yh, dtype=np.float32)
    b = np.asarray(bias, dtype=np.float32)

    nc = _get_program()
    in_maps = make_in_maps(x, whh, wyh, b)
    res = bass_utils.run_bass_kernel_spmd(nc, in_maps,
                                          core_ids=list(range(NCORES)))

    _, _, _, _, _, qcell = _host_indices()
    out = np.empty((B, C, H, W), dtype=np.float32)
    qidx = qcell[None, :, :] + np.arange(BL)[:, None, None]
    for c in range(NCORES):
        ydev = res.results[c]["y"]                   # [512, 4096]
        out[c * BL:(c + 1) * BL] = ydev[:, qidx].transpose(1, 0, 2, 3)
    return out


# revision 15
# speedup vs baseline: 1.3526x; 1.0262x over previous
"""Trainium2 Bass kernel for the SE-sweep DAG-RNN (nn_DAG_RNN_se).

Reference semantics (B=32, C=512, H=W=32):
    h[i,j] = relu(x[:,:,i,j] + (h[i-1,j] + h[i,j-1]) @ W_hh)     # [B, C]
    y[i,j] = h[i,j] @ W_yh + bias

Strategy:
  * Data-parallel over batch: 8 cores x 4 batch elements, zero communication.
  * Anti-diagonal wavefront inside a core: diagonal d holds n_d cells; all
    cells of a diagonal are batched into one set of matmuls.
  * State layout is transposed: h^T [C(4x128 partitions), n_d*B_local] so
    W_hh chunks are the stationary matmul operand; N = 4*n_d <= 128.
  * The +x is folded into the PSUM accumulation via an identity matmul
    (start=True writes x, the 4 W-matmuls accumulate), so the scalar engine
    relu reads PSUM directly and the vector engine only builds the
    neighbour-sum h_sum via free-dim shifted adds (cells on a diagonal are
    ordered by row; neighbours on the next diagonal are adjacent slots).
  * fp16 state + weights (1 cyc/row on PE like bf16, ~8x finer mantissa);
    PSUM accumulates fp32.
  * Output transform y = h @ W_yh in [128,512]-wide matmuls per 512-column
    chunk of the hidden buffer, interleaved into PE bubbles as chunks
    complete.

The full (unsharded) numpy contract is `kernel(**inputs)`; the Bass program
is built and compiled once and cached at module level.
"""

import sys

if "/opt/trn_rl_repo" not in sys.path:
    sys.path.insert(0, "/opt/trn_rl_repo")

import numpy as np

import concourse.bass as bass
import concourse.mybir as mybir
import concourse.tile as tile
from concourse import bacc
from concourse import bass_utils

# ---------------------------------------------------------------- constants
B, C, H, W = 32, 512, 32, 32
NCORES = 8
BL = B // NCORES            # local batch per core = 4
ND = H + W - 1              # 63 diagonals
CT = 4                      # channel chunks of 128
P = 128
YCH = 8                     # output column chunks of 512

F32 = mybir.dt.float32
F16 = mybir.dt.float16
ALU = mybir.AluOpType
ACTF = mybir.ActivationFunctionType

N_D = [min(d, H - 1) - max(0, d - (W - 1)) + 1 for d in range(ND)]
IMIN = [max(0, d - (W - 1)) for d in range(ND)]
OFFB = [0] * (ND + 1)
for _d in range(ND):
    OFFB[_d + 1] = OFFB[_d] + N_D[_d] * BL
TOT = OFFB[ND]              # 4096 columns per chunk row

# y output chunks (col0, width). First 512 columns in 128-wide slivers (they
# become ready early, filling PE gaps in the expanding triangle where the
# per-diagonal matmuls are tiny); then 512-wide; the last 512 split in two so
# the forced serial tail after the final diagonal is half as long.
YCHUNKS = ([(i * 128, 128) for i in range(4)]
           + [(i * 512, 512) for i in range(1, YCH - 1)]
           + [(3584, 256), (3840, 256)])
YREADY = [min(d for d in range(ND) if OFFB[d + 1] >= c0 + w)
          for (c0, w) in YCHUNKS]
YLAG = 2


def _build_program():
    nc = bacc.Bacc("TRN2", target_bir_lowering=False, debug=False,
                   num_devices=NCORES)

    xs = nc.dram_tensor("xs", [P, CT * TOT], F16, kind="ExternalInput").ap()
    whh = nc.dram_tensor("whh", [C, C], F16, kind="ExternalInput").ap()
    wyh = nc.dram_tensor("wyh", [C, C], F16, kind="ExternalInput").ap()
    ident = nc.dram_tensor("ident", [P, P], F16, kind="ExternalInput").ap()
    biasp = nc.dram_tensor("biasp", [P, CT], F32, kind="ExternalInput").ap()
    y = nc.dram_tensor("y", [C, TOT], F32, kind="ExternalOutput").ap()

    with tile.TileContext(nc) as tc:
        with (
            tc.tile_pool(name="persist", bufs=1) as persist,
            tc.tile_pool(name="hspool", bufs=4) as hspool,
            tc.tile_pool(name="ypool", bufs=4) as ypool,
            tc.tile_pool(name="recps", bufs=7, space="PSUM") as recps,
            tc.tile_pool(name="yps", bufs=1, space="PSUM") as yps,
        ):
            # ---- resident tensors ----
            whh_sb = persist.tile([P, CT * C], F16, name="whh_sb")
            wyh_sb = persist.tile([P, CT * C], F16, name="wyh_sb")
            id_sb = persist.tile([P, P], F16, name="id_sb")
            bias_sb = persist.tile([P, CT], F32, name="bias_sb")
            # hidden state, chunk-major: chunk k occupies cols [k*TOT,(k+1)*TOT)
            hj = persist.tile([P, CT * TOT], F16, name="hj")
            # full input, resident: col q = CT*OFFB[d] + ct*(n_d*BL) + s*BL + b
            xsb = persist.tile([P, CT * TOT], F16, name="xsb")

            # Startup ordering matters: the first diagonals need (in order)
            # a small x prefix, the identity, and W_hh. W_yh/bias are not
            # needed until the first y chunk (~diag 18). Spread across the
            # sync (HWDGE) and gpsimd (SWDGE) queues for parallelism.
            nc.sync.dma_start(xsb[:, 0:1024], xs[:, 0:1024])
            nc.sync.dma_start(id_sb[:], ident[:])
            for k in range(CT):
                nc.sync.dma_start(whh_sb[:, k * C:(k + 1) * C],
                                  whh[k * P:(k + 1) * P, :])
            NXD = 6
            w = (CT * TOT - 1024) // NXD
            for j in range(NXD):
                c0 = 1024 + j * w
                c1 = CT * TOT if j == NXD - 1 else c0 + w
                eng = nc.sync if j % 2 == 0 else nc.gpsimd
                eng.dma_start(xsb[:, c0:c1], xs[:, c0:c1])
            for k in range(CT):
                nc.gpsimd.dma_start(wyh_sb[:, k * C:(k + 1) * C],
                                    wyh[k * P:(k + 1) * P, :])
            nc.gpsimd.dma_start(bias_sb[:], biasp[:])

            def w_slice(wsb, k, ct):
                return wsb[:, k * C + ct * P: k * C + ct * P + P]

            def hjs(k, c0, w):
                """h chunk-k cols [c0, c0+w) as an AP."""
                return hj[:, k * TOT + c0: k * TOT + c0 + w]

            def hj2(kbase, c0, w):
                """strided pair view: chunks kbase,kbase+1, cols [c0,c0+w)."""
                pair = hj[:, kbase * TOT:(kbase + 2) * TOT]
                return pair.rearrange("p (k q) -> p k q", k=2)[:, :, c0:c0 + w]

            y_emitted = [False] * len(YCHUNKS)

            def emit_y_chunk(ch):
                c0, wd = YCHUNKS[ch]
                for ct in range(CT):
                    psy = yps.tile([P, 512], F32, tag="psy",
                                   name=f"psy{ch}_{ct}")
                    for k in range(CT):
                        nc.tensor.matmul(
                            psy[:, 0:wd],
                            lhsT=w_slice(wyh_sb, k, ct),
                            rhs=hjs(k, c0, wd),
                            start=(k == 0), stop=(k == CT - 1))
                    ysb = ypool.tile([P, 512], F32, tag="ysb",
                                     name=f"ysb{ch}_{ct}")
                    nc.scalar.activation(ysb[:, 0:wd], psy[:, 0:wd],
                                         ACTF.Identity,
                                         bias=bias_sb[:, ct:ct + 1],
                                         scale=1.0)
                    nc.sync.dma_start(y[ct * P:(ct + 1) * P, c0:c0 + wd],
                                      ysb[:, 0:wd])

            hs_prev = None     # list of 2 pair tiles [P, 2*N]
            for d in range(ND):
                n = N_D[d]
                N = n * BL
                x0 = CT * OFFB[d]

                if d + 1 < ND:
                    N2 = N_D[d + 1] * BL
                    hs_next = [hspool.tile([P, 2 * N2], F16, tag=f"hsp{pr}",
                                           name=f"hsp{pr}_{d + 1}")
                               for pr in range(2)]
                else:
                    hs_next = None

                if d == 0:
                    # h = relu(x): two pair-strided activations
                    for pr in range(2):
                        xv = xsb[:, x0 + 2 * pr * N: x0 + (2 * pr + 2) * N]
                        xv = xv.rearrange("p (k q) -> p k q", k=2)
                        nc.scalar.activation(hj2(2 * pr, OFFB[d], N), xv,
                                             ACTF.Relu)
                else:
                    psg = [recps.tile([P, 512], F32, tag="ps",
                                      name=f"ps{d}_{g}")
                           for g in range(CT)]
                    # early sub-burst: x (identity) + pair-A h_sum chunks.
                    # pair-B chunks of the previous diagonal finish ~1us
                    # later, so deferring their matmuls decouples the PE
                    # stream from the h_sum production chain.
                    for g in range(CT):
                        nc.tensor.matmul(psg[g][:, 0:N], lhsT=id_sb[:],
                                         rhs=xsb[:, x0 + g * N:
                                                 x0 + (g + 1) * N],
                                         start=True, stop=False)
                        for k in (0, 1) if g % 2 == 0 else (1, 0):
                            nc.tensor.matmul(
                                psg[g][:, 0:N],
                                lhsT=w_slice(whh_sb, k, g),
                                rhs=hs_prev[0][:, k * N:(k + 1) * N],
                                start=False, stop=False)
                    # late sub-burst: pair-B chunks, stop on the last;
                    # relu per chunk right after its stop, alternating
                    # engines so the two relus of a pair run concurrently
                    for g in range(CT):
                        ks = (2, 3) if g % 2 == 0 else (3, 2)
                        for idx, k in enumerate(ks):
                            nc.tensor.matmul(
                                psg[g][:, 0:N],
                                lhsT=w_slice(whh_sb, k, g),
                                rhs=hs_prev[1][:, (k - 2) * N:
                                               (k - 1) * N],
                                start=False, stop=(idx == 1))
                        if g % 2 == 0:
                            nc.vector.tensor_scalar_max(
                                hjs(g, OFFB[d], N), psg[g][:, 0:N], 0.0)
                        else:
                            nc.scalar.activation(hjs(g, OFFB[d], N),
                                                 psg[g][:, 0:N], ACTF.Relu)

                # h_sum for diag d+1 from h (pair-strided shifted adds)
                if hs_next is not None:
                    for pr in range(2):
                        hs = hs_next[pr]
                        hsv = hs.rearrange("p (k q) -> p k q", k=2)
                        if d + 1 <= W - 1:
                            # expanding: n2 = n+1
                            nc.vector.tensor_scalar_add(
                                hsv[:, :, 0:BL],
                                hj2(2 * pr, OFFB[d], BL), 0.0)
                            nc.vector.tensor_scalar_add(
                                hsv[:, :, n * BL:(n + 1) * BL],
                                hj2(2 * pr, OFFB[d] + (n - 1) * BL, BL), 0.0)
                            if n > 1:
                                nc.vector.scalar_tensor_tensor(
                                    out=hsv[:, :, BL:n * BL],
                                    in0=hj2(2 * pr, OFFB[d], (n - 1) * BL),
                                    scalar=0.0, op0=ALU.bypass, op1=ALU.add,
                                    in1=hj2(2 * pr, OFFB[d] + BL,
                                            (n - 1) * BL))
                        else:
                            # contracting: n2 = n-1; hs[s] = h[s] + h[s+1]
                            nc.vector.scalar_tensor_tensor(
                                out=hsv[:, :, 0:(n - 1) * BL],
                                in0=hj2(2 * pr, OFFB[d], (n - 1) * BL),
                                scalar=0.0, op0=ALU.bypass, op1=ALU.add,
                                in1=hj2(2 * pr, OFFB[d] + BL, (n - 1) * BL))

                hs_prev = hs_next

                for ch in range(len(YCHUNKS)):
                    if not y_emitted[ch] and YREADY[ch] + YLAG <= d:
                        y_emitted[ch] = True
                        emit_y_chunk(ch)

            for ch in range(len(YCHUNKS)):
                if not y_emitted[ch]:
                    y_emitted[ch] = True
                    emit_y_chunk(ch)

    nc.compile()
    return nc


_CACHE = {}


def _get_program():
    if "nc" not in _CACHE:
        _CACHE["nc"] = _build_program()
    return _CACHE["nc"]


def _host_indices():
    """Precompute gather indices for host-side pre/post permutation."""
    if "idx" in _CACHE:
        return _CACHE["idx"]
    ct_of = np.empty(CT * TOT, dtype=np.int64)
    cell_of = np.empty(CT * TOT, dtype=np.int64)
    b_of = np.empty(CT * TOT, dtype=np.int64)
    cell_base = 0
    for d in range(ND):
        n = N_D[d]
        q0 = CT * OFFB[d]
        blk = n * BL
        for ct in range(CT):
            qs = q0 + ct * blk
            idx = np.arange(blk)
            ct_of[qs:qs + blk] = ct
            cell_of[qs:qs + blk] = cell_base + idx // BL
            b_of[qs:qs + blk] = idx % BL
        cell_base += n
    ci = np.empty(H * W, dtype=np.int64)
    cj = np.empty(H * W, dtype=np.int64)
    qcell = np.empty((H, W), dtype=np.int64)
    cell_base = 0
    for d in range(ND):
        for s in range(N_D[d]):
            i = IMIN[d] + s
            ci[cell_base] = i
            cj[cell_base] = d - i
            qcell[i, d - i] = OFFB[d] + s * BL
            cell_base += 1
    _CACHE["idx"] = (ct_of, cell_of, b_of, ci, cj, qcell)
    return _CACHE["idx"]


def make_in_maps(x, whh, wyh, b):
    ct_of, cell_of, b_of, ci, cj, qcell = _host_indices()
    whh16 = whh.astype(np.float16)
    wyh16 = wyh.astype(np.float16)
    id16 = np.eye(P, dtype=np.float16)
    biasp = np.ascontiguousarray(b.reshape(CT, P).T.astype(np.float32))
    xg = x[:, :, ci, cj]                             # [B, C, 1024]
    in_maps = []
    for c in range(NCORES):
        arr = xg[c * BL:(c + 1) * BL]                # [BL, C, 1024]
        arr3 = arr.reshape(BL, CT, P, H * W).transpose(2, 1, 3, 0)
        xs_core = np.ascontiguousarray(
            arr3[np.arange(P)[:, None], ct_of[None, :], cell_of[None, :],
                 b_of[None, :]].astype(np.float16))
        in_maps.append({"xs": xs_core, "whh": whh16, "wyh": wyh16,
                        "ident": id16, "biasp": biasp})
    return in_maps


def kernel(input, weight_hh, weight_yh, bias):
    x = np.ascontiguousarray(np.asarray(input, dtype=np.float32))
    whh = np.asarray(weight_hh, dtype=np.float32)
    wyh = np.asarray(weight_yh, dtype=np.float32)
    b = np.asarray(bias, dtype=np.float32)

    nc = _get_program()
    in_maps = make_in_maps(x, whh, wyh, b)
    res = bass_utils.run_bass_kernel_spmd(nc, in_maps,
                                          core_ids=list(range(NCORES)))

    _, _, _, _, _, qcell = _host_indices()
    out = np.empty((B, C, H, W), dtype=np.float32)
    qidx = qcell[None, :, :] + np.arange(BL)[:, None, None]
    for c in range(NCORES):
        ydev = res.results[c]["y"]                   # [512, 4096]
        out[c * BL:(c + 1) * BL] = ydev[:, qidx].transpose(1, 0, 2, 3)
    return out
